# revision 12
# baseline (speedup 1.0000x reference)
"""Trainium2 Bass kernel for nn_BertSelfAttention_79448305042103.

Two independent quantized BERT self-attention branches (B=8, S=512, H=768,
NH=12), 8-bit symmetric activation quant (layerwise scales) + 1-bit BWN
weights.

Sharding (8 NeuronCores): pair-sharding — core c owns batch c of BOTH
branches (unit 0 = branch1, unit 1 = branch2). Every collective (tiny
scalar AllReduce-max over all 8 cores) hides behind the other unit's
compute. Weights are host-prepacked: sign(W) as transposed bf16 (+/-1) and
alpha = mean|W| scalars (pure weight pre-pack; all activation statistics
are computed on device).

Numerics: all matmuls run in bf16 on small-integer data (|int| <= 127,
sign(W) in {-1,+1}) with exact fp32 PSUM accumulation. Dequant scales fold
into the next quant scale. Softmax is a SINGLE pass: scores [q,k] -> ACT
exp (scale=s_sc) with accum_out giving row sums d[q]; row maxes (DVE)
feed the global prob max -> s_p via AllReduce. The division by d, the
multiply by s_p AND the [q,k]->[k,q] transpose all happen in ONE PE matmul
per 128x128 tile: out = e_blk^T @ diag(s_p/d[q]). A single dual-op
(+M,-M magic) DVE/Pool pass rounds psum->sbuf bf16. The context matmul
packs 2 heads per PSUM tile via column tiling; output is ctx^T,
un-transposed on the host.

SBUF pressure: raw projections spill PSUM->DRAM and reload fused into the
quant chunks; h is re-read from DRAM for quantization; e tiles are
per-head-pair and shared between units by tag reuse (unit 1's exp starts
in each slot as unit 0's tail frees it); pqT and diag are small rotating
buffers. Emission order is hand-interleaved per engine FIFO.
"""
import sys
sys.path.insert(0, '/opt/trn_rl_repo')

import numpy as np

B, S, H, NH = 8, 512, 768, 12
DH = H // NH
CLIP = 2.5
QMAX = 127.0
MAGIC = 12582912.0  # 1.5*2^23: ((x+M)-M) == round-half-even(x) for |x| < 2^22

_CACHE = {}
LAST_RESULT = None


def build(s, h, nh):
    import concourse.bass as bass
    import concourse.mybir as mybir
    import concourse.tile as tile
    from concourse import bacc, bass_isa
    from concourse.masks import make_identity
    from contextlib import ExitStack

    F32 = mybir.dt.float32
    BF16 = mybir.dt.bfloat16
    AT = mybir.ActivationFunctionType
    OP = mybir.AluOpType
    dh = h // nh
    it = h // 128          # 6 row blocks
    tt = s // 128          # 4 seq blocks
    hp = nh // 2           # 6 head pairs
    ncol = nh * tt         # 48 (d / rowmax columns per unit)
    groups = [[0, 1, 2, 3, 4, 5, 6, 7]]
    wnames = ['q', 'k', 'v']
    U = 2

    nc = bacc.Bacc(None, target_bir_lowering=False, debug=False)

    hT = nc.declare_dram_parameter("hT", [U, h, s], F32, isOutput=False)
    Wt = {(u, w): nc.declare_dram_parameter(f"W{w}T{u}", [h, h], BF16,
                                            isOutput=False)
          for u in range(U) for w in wnames}
    alphas = nc.declare_dram_parameter("alphas", [1, 6], F32, isOutput=False)
    ctxT = nc.declare_dram_parameter("ctxT", [U, h, s], F32, isOutput=True)

    # DRAM scratch for raw projection spills
    rawqk_d = nc.dram_tensor("rawqk_d", [U, 2, h, s], F32)

    # AR1 [h0,h1]; AR2 [q0,k0]; AR3 [q1,k1]; AR4 [p0,v0,v1]; AR5 [p1]
    cc_n = {'h': 2, 'qk0': 2, 'qk1': 2, 'pv': 3, 'p1': 1}
    cc_in = {n: nc.dram_tensor(f"cc_in_{n}", [1, c], F32)
             for n, c in cc_n.items()}
    cc_out = {n: nc.dram_tensor(f"cc_out_{n}", [1, c], F32)
              for n, c in cc_n.items()}

    with tile.TileContext(nc) as tc, ExitStack() as es:
        scal = es.enter_context(tc.tile_pool(name="scal", bufs=1))
        persist = es.enter_context(tc.tile_pool(name="persist", bufs=1))

        def allreduce(name, srcs):
            for i, sp in enumerate(srcs):
                nc.sync.dma_start(out=cc_in[name].ap()[0:1, i:i + 1], in_=sp)
            nc.gpsimd.collective_compute(
                "AllReduce", OP.max, replica_groups=groups,
                ins=[cc_in[name].ap()], outs=[cc_out[name].ap()])
            g = scal.tile([1, cc_n[name]], F32, tag=f"cc_{name}",
                          name=f"cc_{name}")
            nc.sync.dma_start(out=g, in_=cc_out[name].ap())
            return g

        ident = persist.tile([128, 128], BF16, tag="ident")
        make_identity(nc, ident)

        # ---- long-lived pools (bottom of the LIFO pool stack) ----
        pool_e = es.enter_context(tc.tile_pool(name="ep", bufs=1))
        pool_pq = es.enter_context(tc.tile_pool(name="pqp", bufs=2))
        pool_dg = es.enter_context(tc.tile_pool(name="dgp", bufs=2))
        pool_qiv = es.enter_context(tc.tile_pool(name="qivp", bufs=1))
        pool_out = es.enter_context(tc.tile_pool(name="outp", bufs=2))
        pool_ck = es.enter_context(tc.tile_pool(name="ckp", bufs=2))
        # all chunk/bounce buffers share one [128,s] f32 tag
        ps_sc = es.enter_context(tc.tile_pool(name="ps_sc", bufs=3,
                                              space="PSUM"))
        # ---- mid-life pools, closed in LIFO order during the kernel ----
        es_qk = ExitStack()
        pool_qk = es_qk.enter_context(tc.tile_pool(name="qkp", bufs=1))
        es_rv = ExitStack()
        pool_rv = es_rv.enter_context(tc.tile_pool(name="rvp", bufs=1))
        es_wv = ExitStack()
        pool_wv = es_wv.enter_context(tc.tile_pool(name="wvp", bufs=1))
        es_pr = ExitStack()
        ps_pr = es_pr.enter_context(tc.tile_pool(name="ps_pr", bufs=4,
                                                 space="PSUM"))
        es_wqk = ExitStack()
        pool_wqk = es_wqk.enter_context(tc.tile_pool(name="wqkp", bufs=1))

        # weight DMA (starts immediately)
        sw = {}
        for u in range(U):
            for w in wnames:
                pool = pool_wv if w == 'v' else pool_wqk
                t = pool.tile([128, it, h], BF16, tag=f"sw{u}{w}",
                              name=f"sw{u}{w}")
                for i in range(it):
                    nc.sync.dma_start(
                        out=t[:, i, :],
                        in_=Wt[(u, w)].ap()[128 * i:128 * (i + 1), :])
                sw[(u, w)] = t
        al_sb = scal.tile([1, 6], F32, tag="alphas")
        nc.sync.dma_start(out=al_sb, in_=alphas.ap())

        # ---------------- S0: h absmax (chunked), AR1, h quant ----------------
        nch = it  # one 128-row block (512 f32 cols/partition) per chunk
        hmx = scal.tile([128, U * nch], F32, tag="hmx")
        for u in range(U):
            hTv = hT.ap()[u].rearrange("(b a) c -> a b c", a=128)
            for c in range(nch):
                ck = pool_ck.tile([128, s], F32, tag="ck")
                nc.sync.dma_start(out=ck, in_=hTv[:, c, :])
                nc.vector.tensor_reduce(out=hmx[:, u * nch + c:u * nch + c + 1],
                                        in_=ck, axis=mybir.AxisListType.X,
                                        op=OP.max, apply_absolute_value=True)
        hmax_p = []
        for u in range(U):
            col = scal.tile([128, 1], F32, tag=f"hmaxc{u}")
            nc.vector.tensor_reduce(out=col, in_=hmx[:, u * nch:(u + 1) * nch],
                                    axis=mybir.AxisListType.X, op=OP.max)
            p = scal.tile([128, 1], F32, tag=f"hmaxp{u}")
            nc.gpsimd.partition_all_reduce(p, col, channels=128,
                                           reduce_op=bass_isa.ReduceOp.max)
            hmax_p.append(p)
        g_h = allreduce('h', [p[0:1, 0:1] for p in hmax_p])

        s_in_bc, rdsc = [], {}
        for u in range(U):
            mh = scal.tile([1, 1], F32, tag=f"mh{u}")
            nc.vector.tensor_scalar(out=mh, in0=g_h[0:1, u:u + 1],
                                    scalar1=CLIP, scalar2=None, op0=OP.min)
            rmh = scal.tile([1, 1], F32, tag=f"rmh{u}")
            nc.vector.reciprocal(out=rmh, in_=mh)
            si = scal.tile([1, 1], F32, tag=f"si{u}")
            nc.vector.tensor_scalar(out=si, in0=rmh, scalar1=QMAX,
                                    scalar2=None, op0=OP.mult)
            sib = scal.tile([128, 1], F32, tag=f"sib{u}")
            nc.gpsimd.partition_broadcast(sib, si, channels=128)
            s_in_bc.append(sib)
            for wi, w in enumerate(wnames):
                d1 = scal.tile([1, 1], F32, tag=f"dsc{u}{w}")
                nc.vector.tensor_tensor(
                    out=d1, in0=al_sb[0:1, 3 * u + wi:3 * u + wi + 1],
                    in1=mh, op=OP.mult)
                nc.vector.tensor_scalar(out=d1, in0=d1, scalar1=1.0 / QMAX,
                                        scalar2=None, op0=OP.mult)
                rdsc[(u, w)] = d1

        # h re-read + 3-instr quant (clip via int clamp), chunked
        xq = [pool_qk.tile([128, it, s], BF16, tag=f"xq{u}", name=f"xq{u}")
              for u in range(U)]
        for u in range(U):
            eng = nc.vector if u == 0 else nc.gpsimd
            hTv = hT.ap()[u].rearrange("(b a) c -> a b c", a=128)
            for c in range(nch):
                ck = pool_ck.tile([128, s], F32, tag="ck")
                nc.sync.dma_start(out=ck, in_=hTv[:, c, :])
                eng.tensor_scalar(out=ck, in0=ck, scalar1=s_in_bc[u],
                                  scalar2=MAGIC, op0=OP.mult, op1=OP.add)
                eng.tensor_scalar(out=ck, in0=ck, scalar1=MAGIC + QMAX,
                                  scalar2=MAGIC - QMAX, op0=OP.min,
                                  op1=OP.max)
                eng.tensor_scalar(out=xq[u][:, c, :], in0=ck,
                                  scalar1=MAGIC, scalar2=None,
                                  op0=OP.subtract)

        # ---------------- S1: q,k projections (spill raw to DRAM) -----------
        rmaxc = {}
        for u in range(U):
            for w in ['q', 'k']:
                rmaxc[(u, w)] = scal.tile([128, it], F32, tag=f"rmc{u}{w}",
                                          name=f"rmc{u}{w}")
            rmaxc[(u, 'v')] = scal.tile([128, tt * 2], F32, tag=f"rmc{u}v",
                                        name=f"rmc{u}v")

        def proj_qk(u):
            for wi, w in enumerate(['q', 'k']):
                for io in range(it):
                    ps = ps_pr.tile([128, s], F32, tag="ps")
                    for ii in range(it):
                        nc.tensor.matmul(
                            ps, sw[(u, w)][:, ii, 128 * io:128 * (io + 1)],
                            xq[u][:, ii, :],
                            start=(ii == 0), stop=(ii == it - 1))
                    nc.vector.tensor_reduce(
                        out=rmaxc[(u, w)][:, io:io + 1], in_=ps,
                        axis=mybir.AxisListType.X, op=OP.max,
                        apply_absolute_value=True)
                    bn = pool_ck.tile([128, s], F32, tag="ck")
                    nc.scalar.activation(bn, ps, AT.Copy)
                    nc.sync.dma_start(
                        out=rawqk_d.ap()[u, wi, 128 * io:128 * (io + 1), :],
                        in_=bn)

        rawv = [pool_rv.tile([128, tt, h], F32, tag=f"rawv{u}",
                             name=f"rawv{u}") for u in range(U)]

        def proj_v_chunk(u, ts_):
            for no in range(2):
                w0, w1 = (h // 2) * no, (h // 2) * (no + 1)
                psf = ps_pr.tile([128, s], F32, tag="ps")
                ps = psf[:, :h // 2]
                for ii in range(it):
                    nc.tensor.matmul(
                        ps, xq[u][:, ii, 128 * ts_:128 * (ts_ + 1)],
                        sw[(u, 'v')][:, ii, w0:w1],
                        start=(ii == 0), stop=(ii == it - 1))
                nc.vector.tensor_reduce(
                    out=rmaxc[(u, 'v')][:, 2 * ts_ + no:2 * ts_ + no + 1],
                    in_=ps, axis=mybir.AxisListType.X, op=OP.max,
                    apply_absolute_value=True)
                if u == 0:
                    nc.vector.tensor_copy(out=rawv[u][:, ts_, w0:w1], in_=ps)
                else:
                    nc.scalar.activation(rawv[u][:, ts_, w0:w1], ps, AT.Copy)

        def local_max(u, w):
            rm = scal.tile([128, 1], F32, tag=f"rm{u}{w}")
            nc.vector.tensor_reduce(out=rm, in_=rmaxc[(u, w)],
                                    axis=mybir.AxisListType.X, op=OP.max)
            rp = scal.tile([128, 1], F32, tag=f"rp{u}{w}")
            nc.gpsimd.partition_all_reduce(rp, rm, channels=128,
                                           reduce_op=bass_isa.ReduceOp.max)
            return rp

        proj_qk(0)
        g_qk0 = allreduce('qk0', [local_max(0, 'q')[0:1, 0:1],
                                  local_max(0, 'k')[0:1, 0:1]])
        proj_qk(1)
        es_wqk.close()
        g_qk1 = allreduce('qk1', [local_max(1, 'q')[0:1, 0:1],
                                  local_max(1, 'k')[0:1, 0:1]])

        def qscales(u, w, g_ap):
            m = scal.tile([1, 1], F32, tag=f"m{u}{w}")
            nc.vector.tensor_tensor(out=m, in0=g_ap, in1=rdsc[(u, w)],
                                    op=OP.mult)
            nc.vector.tensor_scalar(out=m, in0=m, scalar1=CLIP, scalar2=None,
                                    op0=OP.min)
            rem = scal.tile([1, 1], F32, tag=f"rem{u}{w}")
            nc.vector.reciprocal(out=rem, in_=m)
            st = scal.tile([1, 1], F32, tag=f"st{u}{w}")
            nc.vector.tensor_scalar(out=st, in0=rem, scalar1=QMAX,
                                    scalar2=None, op0=OP.mult)
            se = scal.tile([1, 1], F32, tag=f"se{u}{w}")
            nc.vector.tensor_tensor(out=se, in0=st, in1=rdsc[(u, w)],
                                    op=OP.mult)
            seb = scal.tile([128, 1], F32, tag=f"seb{u}{w}")
            nc.gpsimd.partition_broadcast(seb, se, channels=128)
            return st, seb

        qi, s_t = {}, {}

        def quant_qk(u, g):
            # reload raw chunks from DRAM, 2-instr magic round (no clamp)
            for wi, w in enumerate(['q', 'k']):
                st, seb = qscales(u, w, g[0:1, wi:wi + 1])
                s_t[(u, w)] = st
                dst = pool_qk.tile([128, it, s], BF16, tag=f"qi{u}{w}",
                                   name=f"qi{u}{w}")
                qi[(u, w)] = dst
                eng = nc.vector if w == 'q' else nc.gpsimd
                for io in range(it):
                    ck = pool_ck.tile([128, s], F32, tag="ck")
                    nc.sync.dma_start(
                        out=ck,
                        in_=rawqk_d.ap()[u, wi, 128 * io:128 * (io + 1), :])
                    eng.tensor_scalar(out=ck, in0=ck, scalar1=seb,
                                      scalar2=MAGIC, op0=OP.mult, op1=OP.add)
                    eng.tensor_scalar(out=dst[:, io, :], in0=ck,
                                      scalar1=MAGIC, scalar2=None,
                                      op0=OP.subtract)

        def scores_scale(u):
            t = scal.tile([1, 1], F32, tag=f"tsc{u}")
            nc.vector.tensor_tensor(out=t, in0=s_t[(u, 'q')],
                                    in1=s_t[(u, 'k')], op=OP.mult)
            nc.vector.tensor_scalar(out=t, in0=t, scalar1=float(np.sqrt(dh)),
                                    scalar2=None, op0=OP.mult)
            ssc = scal.tile([1, 1], F32, tag=f"ssc{u}")
            nc.vector.reciprocal(out=ssc, in_=t)
            sscb = scal.tile([128, 1], F32, tag=f"sscb{u}")
            nc.gpsimd.partition_broadcast(sscb, ssc, channels=128)
            return sscb

        quant_qk(0, g_qk0)
        s_sc_bc0 = scores_scale(0)
        quant_qk(1, g_qk1)
        s_sc_bc1 = scores_scale(1)

        # ---------------- S2: unit-0 scores+exp interleaved with v-proj -----
        d_buf = [persist.tile([128, ncol], F32, tag=f"d{u}", name=f"d{u}")
                 for u in range(U)]
        rx_buf = [persist.tile([128, ncol], F32, tag=f"rx{u}", name=f"rx{u}")
                  for u in range(U)]

        def scores_chunk(u, p_, sscb):
            # e tile: tag shared between units -> unit1 reuses unit0's slot
            e_t = pool_e.tile([128, 2, tt, s], BF16, tag=f"ep{p_}",
                              name=f"e{u}p{p_}")
            for parity in range(2):
                hh = 2 * p_ + parity
                lo = 64 * parity
                for t_ in range(tt):
                    ps = ps_sc.tile([128, s], F32, tag="pss")
                    nc.tensor.matmul(
                        ps,
                        qi[(u, 'q')][lo:lo + 64, p_, 128 * t_:128 * (t_ + 1)],
                        qi[(u, 'k')][lo:lo + 64, p_, :],
                        start=True, stop=True, tile_position=(lo, 0))
                    col = hh * tt + t_
                    nc.scalar.activation(
                        e_t[:, parity, t_, :], ps, AT.Exp,
                        scale=sscb, accum_out=d_buf[u][:, col:col + 1])
                    nc.vector.tensor_reduce(
                        out=rx_buf[u][:, col:col + 1],
                        in_=e_t[:, parity, t_, :],
                        axis=mybir.AxisListType.X, op=OP.max)
            return e_t

        e0 = {}
        for p_ in range(hp):
            e0[p_] = scores_chunk(0, p_, s_sc_bc0)
            if p_ < 4:
                proj_v_chunk(0, p_)
            else:
                proj_v_chunk(1, p_ - 4)
        proj_v_chunk(1, 2)
        proj_v_chunk(1, 3)
        es_pr.close()
        es_wv.close()
        vmax0 = local_max(0, 'v')
        vmax1 = local_max(1, 'v')

        rd0 = persist.tile([128, ncol], F32, tag="rd0")
        nc.vector.reciprocal(out=rd0, in_=d_buf[0])
        pr0 = persist.tile([128, ncol], F32, tag="pr0")
        nc.vector.tensor_tensor(out=pr0, in0=rx_buf[0], in1=rd0, op=OP.mult)
        prm0 = scal.tile([128, 1], F32, tag="prm0")
        nc.vector.tensor_reduce(out=prm0, in_=pr0,
                                axis=mybir.AxisListType.X, op=OP.max)
        prp0 = scal.tile([128, 1], F32, tag="prp0")
        nc.gpsimd.partition_all_reduce(prp0, prm0, channels=128,
                                       reduce_op=bass_isa.ReduceOp.max)
        g_pv = allreduce('pv', [prp0[0:1, 0:1], vmax0[0:1, 0:1],
                                vmax1[0:1, 0:1]])

        # ---------------- S3 psum pools ----------------
        es_s3 = ExitStack()
        ps_tr = es_s3.enter_context(tc.tile_pool(name="ps_tr", bufs=3,
                                                 space="PSUM"))
        ps_cx = es_s3.enter_context(tc.tile_pool(name="ps_cx", bufs=2,
                                                 space="PSUM"))

        def s_p_of(u, g_ap):
            rg = scal.tile([1, 1], F32, tag=f"rgp{u}")
            nc.vector.reciprocal(out=rg, in_=g_ap)
            sp = scal.tile([1, 1], F32, tag=f"sp{u}")
            nc.vector.tensor_scalar(out=sp, in0=rg, scalar1=QMAX,
                                    scalar2=None, op0=OP.mult)
            spb = scal.tile([128, 1], F32, tag=f"spb{u}")
            nc.gpsimd.partition_broadcast(spb, sp, channels=128)
            return sp, spb

        def dhat_of(u, spb, rd_ap):
            dh_f = scal.tile([128, ncol], F32, tag=f"dhf{u}")
            nc.vector.tensor_scalar(out=dh_f, in0=rd_ap, scalar1=spb,
                                    scalar2=None, op0=OP.mult)
            dh_b = scal.tile([128, ncol], BF16, tag=f"dhb{u}")
            nc.vector.tensor_copy(out=dh_b, in_=dh_f)
            return dh_b

        def rdqc_of(u, sp):
            t = scal.tile([1, 1], F32, tag=f"dqc{u}")
            nc.vector.tensor_tensor(out=t, in0=sp, in1=s_t[(u, 'v')],
                                    op=OP.mult)
            r = scal.tile([1, 1], F32, tag=f"rdqc{u}")
            nc.vector.reciprocal(out=r, in_=t)
            rb = scal.tile([128, 1], F32, tag=f"rdqcb{u}")
            nc.gpsimd.partition_broadcast(rb, r, channels=128)
            return rb

        def quant_v(u, g_ap, eng):
            st, seb = qscales(u, 'v', g_ap)
            s_t[(u, 'v')] = st
            dst = pool_qiv.tile([128, tt, h], BF16, tag=f"qi{u}v",
                                name=f"qi{u}v")
            qi[(u, 'v')] = dst
            t = rawv[u]
            eng.tensor_scalar(out=t, in0=t, scalar1=seb, scalar2=MAGIC,
                              op0=OP.mult, op1=OP.add)
            eng.tensor_scalar(out=dst, in0=t, scalar1=MAGIC, scalar2=None,
                              op0=OP.subtract)

        def tail_chunk(u, p_, e_t, dh_b, rdqc_bc):
            # per-head-pair diag tiles (small, rotating; Pool = SBUF-only)
            dg = pool_dg.tile([128, 2, tt, 128], BF16, tag="dg",
                              name=f"dg{u}p{p_}")
            nc.gpsimd.tensor_tensor(
                out=dg,
                in0=ident.unsqueeze(1).unsqueeze(1)
                    .broadcast_to([128, 2, tt, 128]),
                in1=dh_b[:, 2 * p_ * tt:(2 * p_ + 2) * tt]
                    .rearrange("a (b c) -> a b c", b=2)
                    .unsqueeze(3).broadcast_to([128, 2, tt, 128]),
                op=OP.mult)
            pq = pool_pq.tile([128, 2, tt, s], BF16, tag="pqT",
                              name=f"pqT{u}p{p_}")
            for parity in range(2):
                for kb in range(tt):
                    ps = ps_tr.tile([128, s], F32, tag="pst")
                    for tq in range(tt):
                        nc.tensor.matmul(
                            ps[:, 128 * tq:128 * (tq + 1)],
                            e_t[:, parity, tq, 128 * kb:128 * (kb + 1)],
                            dg[:, parity, tq, :],
                            start=True, stop=True, skip_group_check=True)
                    nc.vector.tensor_scalar(out=pq[:, parity, kb, :],
                                            in0=ps, scalar1=MAGIC,
                                            scalar2=MAGIC, op0=OP.add,
                                            op1=OP.subtract)
            psc = ps_cx.tile([128, s], F32, tag="psc")
            for t_ in range(tt):
                for parity in range(2):
                    hh = 2 * p_ + parity
                    nc.tensor.matmul(
                        psc[64 * parity:64 * parity + 64, :],
                        qi[(u, 'v')][:, t_, dh * hh:dh * (hh + 1)],
                        pq[:, parity, t_, :],
                        start=(t_ == 0), stop=(t_ == tt - 1),
                        tile_position=(0, 64 * parity),
                        skip_group_check=True)
            o = pool_out.tile([128, s], F32, tag="o")
            nc.scalar.activation(o, psc, AT.Copy, scale=rdqc_bc)
            nc.sync.dma_start(out=ctxT.ap()[u, 128 * p_:128 * (p_ + 1), :],
                              in_=o)

        # post-AR4 scalars + v quants
        sp0, spb0 = s_p_of(0, g_pv[0:1, 0:1])
        dhb0 = dhat_of(0, spb0, rd0)
        quant_v(0, g_pv[0:1, 1:2], nc.vector)
        quant_v(1, g_pv[0:1, 2:3], nc.gpsimd)
        es_rv.close()
        rdqc0 = rdqc_of(0, sp0)

        # unit-1 prob ratio, per chunk on DVE (also assembles rd1)
        pr1 = persist.tile([128, ncol], F32, tag="pr1")
        rd1 = persist.tile([128, ncol], F32, tag="rd1")

        def pr1_chunk(p_):
            c0, c1 = (2 * p_) * tt, (2 * p_ + 2) * tt
            nc.vector.reciprocal(out=rd1[:, c0:c1], in_=d_buf[1][:, c0:c1])
            nc.vector.tensor_tensor(out=pr1[:, c0:c1],
                                    in0=rx_buf[1][:, c0:c1],
                                    in1=rd1[:, c0:c1], op=OP.mult)

        # ---- interleaved: unit-0 tail frees each e slot for unit-1 scores
        e1 = {}
        for p_ in range(hp):
            tail_chunk(0, p_, e0[p_], dhb0, rdqc0)
            e1[p_] = scores_chunk(1, p_, s_sc_bc1)
            pr1_chunk(p_)

        prm1 = scal.tile([128, 1], F32, tag="prm1")
        nc.vector.tensor_reduce(out=prm1, in_=pr1,
                                axis=mybir.AxisListType.X, op=OP.max)
        prp1 = scal.tile([128, 1], F32, tag="prp1")
        nc.gpsimd.partition_all_reduce(prp1, prm1, channels=128,
                                       reduce_op=bass_isa.ReduceOp.max)
        g_p1 = allreduce('p1', [prp1[0:1, 0:1]])

        sp1, spb1 = s_p_of(1, g_p1[0:1, 0:1])
        dhb1 = dhat_of(1, spb1, rd1)
        rdqc1 = rdqc_of(1, sp1)
        for p_ in range(hp):
            tail_chunk(1, p_, e1[p_], dhb1, rdqc1)

        es_s3.close()
        es_qk.close()

    nc.compile()
    return nc


def _get_nc():
    key = (S, H, NH)
    if key not in _CACHE:
        _CACHE[key] = build(S, H, NH)
    return _CACHE[key]


def _ensure_profile_hook():
    """bass_utils imports antenv.axon_hooks when tracing; this image's antenv
    lacks it. Inject a minimal implementation backed by libaxon_pjrt.so."""
    import importlib
    import os
    import types
    try:
        importlib.import_module('antenv.axon_hooks')
        return
    except ImportError:
        pass
    import antenv
    mod = types.ModuleType('antenv.axon_hooks')
    mod._hook = None

    def set_axon_ntff_profile_hook(h):
        mod._hook = h

    def get_axon_ntff_profile_hook():
        return mod._hook

    mod.set_axon_ntff_profile_hook = set_axon_ntff_profile_hook
    mod.get_axon_ntff_profile_hook = get_axon_ntff_profile_hook
    sys.modules['antenv.axon_hooks'] = mod
    antenv.axon_hooks = mod

    so_path = '/opt/axon/libaxon_pjrt.so'
    if os.path.exists(so_path):
        try:
            sys.path.insert(0, '/root/.axon_site')
            from trn_agent_boot.trn_boot import _ntff_profile_via_ctypes
            mod._hook = _ntff_profile_via_ctypes(so_path)
        except Exception:
            mod._hook = None


def kernel(**inputs):
    import os
    import ml_dtypes
    from concourse.bass_utils import run_bass_kernel_spmd
    if os.environ.get('BASS_TRACE'):
        _ensure_profile_hook()

    nc = _get_nc()
    hs = [np.asarray(inputs['hidden_states1'], np.float32),
          np.asarray(inputs['hidden_states2'], np.float32)]
    for br in range(2):
        m = np.asarray(inputs[f'attention_mask{br}'], np.float32)
        assert not np.any(m), "nonzero attention masks not supported"

    WsT = {}
    alphas = np.empty((1, 6), np.float32)
    for u in range(2):
        for wi, w in enumerate(['q', 'k', 'v']):
            W = np.asarray(inputs[f'W{w}{u + 1}'], np.float32)
            alphas[0, 3 * u + wi] = np.abs(W).mean()
            WsT[(u, w)] = np.ascontiguousarray(
                np.sign(W).T.astype(ml_dtypes.bfloat16))

    in_maps = []
    for c in range(8):
        hTc = np.stack([np.ascontiguousarray(hs[0][c].T),
                        np.ascontiguousarray(hs[1][c].T)])
        im = {'hT': hTc, 'alphas': alphas}
        for u in range(2):
            for w in ['q', 'k', 'v']:
                im[f'W{w}T{u}'] = WsT[(u, w)]
        in_maps.append(im)

    global LAST_RESULT
    res = run_bass_kernel_spmd(nc, in_maps, core_ids=list(range(8)))
    LAST_RESULT = res

    outs = []
    for br in range(2):
        ctx = np.empty((B, S, H), np.float32)
        for c in range(8):
            ctx[c] = res.results[c]['ctxT'][br].T
        outs.append(ctx)
    return outs[0], outs[1]
